# revision 9
# baseline (speedup 1.0000x reference)
"""Trainium2 Bass kernel for the CNN-VAE loss:

    prob = einsum('klb,hwb->klhw', beta, A) * 5000
    mse  = mean(sum(|x - prob[:, :, None]|^2, axis=1))

Strategy (v2: algebraic expansion, bf16 stream)
-----------------------------------------------
Expand  sum |x - p|^2 = sum x^2 - 2*sum x*p + C*sum p^2  (p broadcast over
the C=3 channel dim).  With p = SCALE * einsum('klb,hwb', beta, A):

  T1 = sum x^2                                   -> device (per-partition
       accumulators; split between ACT Square+accum and DVE fused
       multiply-reduce so neither engine exceeds the DMA window)
  T2 = -2*SCALE * sum_b sum_pix A[pix,b]*Y[b,pix],
       Y[b,pix] = sum_{kl,c} beta[kl,b] * x[kl,c,pix]
       -> Y computed on the otherwise-idle PE: beta (128x3 bf16) is the
       stationary operand, x streams through as the moving operand, and
       the c-sum is folded into the PSUM accumulation (3 matmuls/chunk).
       The A-weighted pixel reduction is one fused DVE
       tensor_tensor_reduce straight out of PSUM.
  T3 = C*SCALE^2 * sum_kl beta^T (A^T A) beta   -> host, f64, from the
       tiny beta/A inputs (A^T A is 3x3 over 40k pixels).

x is uploaded as bf16 (halves HBM traffic; contributes <1e-9 relative
error vs the 2e-2 tolerance since T1+T2 are ~1e-7 of the total).  The
hw axis is sharded across the 8 cores (5000 pixels each); every core
sees all 128 (k,l) rows.  beta/A constants go out on the scalar-engine
HWDGE queue so they land before the x stream monopolizes the SDMA
engines.
"""

import numpy as np

K, L, NB, H, W = 16, 8, 3, 200, 200
KL = K * L          # 128 partitions
C = 3               # broadcast channel dim of x
HW = H * W          # 40000
N_CORES = 8
HW_SHARD = HW // N_CORES   # 5000
GROUP = 1000               # pixels per steady-state iteration
NG = HW_SHARD // GROUP     # 5
HALF = GROUP // 2          # 500: matmul free-dim chunk (fits one PSUM bank)
BANK = 512                 # PSUM bank width in f32
XCOLS = C * GROUP          # 3000 x columns per group
DVE_SQ = 1152              # x^2 columns handled by DVE (rest on ACT)
SCALE = 5000.0
DENOM = float(K * C * H * W)  # mean denominator (sum over L folded in)

_NC = None


def _build():
    global _NC
    if _NC is not None:
        return _NC
    from contextlib import ExitStack

    import concourse.bacc as bacc
    import concourse.mybir as mybir
    import concourse.tile as tile

    f32 = mybir.dt.float32
    bf16 = mybir.dt.bfloat16
    nc = bacc.Bacc("TRN2", target_bir_lowering=False, debug=False)

    xg = nc.dram_tensor("xg", [NG, KL, C, GROUP], bf16, kind="ExternalInput").ap()
    bsb = nc.dram_tensor("bsb", [KL, NB], bf16, kind="ExternalInput").ap()
    asb = nc.dram_tensor("asb", [NB, HW_SHARD], bf16, kind="ExternalInput").ap()
    out_sq = nc.dram_tensor("out_sq", [KL, 2 * NG], f32, kind="ExternalOutput").ap()
    out_t2 = nc.dram_tensor("out_t2", [NB, 2 * NG], f32, kind="ExternalOutput").ap()

    with tile.TileContext(nc) as tc, ExitStack() as ctx:
        const = ctx.enter_context(tc.tile_pool(name="const", bufs=1))
        xpool = ctx.enter_context(tc.tile_pool(name="x", bufs=3))
        spool = ctx.enter_context(tc.tile_pool(name="scr", bufs=2))
        ppool = ctx.enter_context(tc.tile_pool(name="psum", bufs=3, space="PSUM"))
        wpool = ctx.enter_context(tc.tile_pool(name="wps", bufs=1, space="PSUM"))

        # constants on the GPSIMD SWDGE queue: separate issue path from the
        # x stream so they land immediately
        b_sb = const.tile([KL, NB], bf16)
        nc.gpsimd.dma_start(b_sb[:], bsb[:])
        a_sb = const.tile([NB, HW_SHARD], bf16)
        nc.gpsimd.dma_start(a_sb[:], asb[:])

        acc_sq = const.tile([KL, 2 * NG], f32)
        acc_t2 = const.tile([NB, 2 * NG], f32)

        # warm the ACT Square spline table while DMAs are in flight
        warm = const.tile([KL, 8], f32)
        nc.vector.memset(warm[:], 0.0)
        nc.scalar.activation(warm[:], warm[:], mybir.ActivationFunctionType.Square)

        # warm the PE HAM clock gate (idle default is 1.2 GHz; ~3.4us of
        # activity lifts it to 2.4 GHz before the real matmuls arrive)
        wmm = const.tile([KL, BANK], bf16)
        nc.vector.memset(wmm[:], 0.0)
        ydum = wpool.tile([NB, BANK], f32)
        for _ in range(8):
            nc.tensor.matmul(
                ydum[:, :BANK], wmm[:, :NB], wmm[:], start=True, stop=True
            )

        # timing probes (outputs unused; host ignores them)
        probe = const.tile([KL, 64], f32)
        pscr = spool.tile([KL, XCOLS], bf16)

        for g in range(NG):
            xt = xpool.tile([KL, C, GROUP], bf16)
            nc.sync.dma_start(xt[:], xg[g])

            # PE: Y[b, pix] += sum_kl beta[kl,b] * x[kl,c,pix], c folded
            # into the PSUM accumulation group
            yt = ppool.tile([NB, 2 * BANK], f32)
            for h in range(2):
                for c in range(C):
                    nc.tensor.matmul(
                        yt[:, h * BANK : h * BANK + HALF],
                        b_sb[:],
                        xt[:, c, h * HALF : (h + 1) * HALF],
                        start=(c == 0),
                        stop=(c == C - 1),
                    )

            # T2 partial: acc_t2[:, 2g+h] = sum_pix Y * A^T (fused mult+accum,
            # scalar_tensor_tensor reads Y straight out of PSUM)
            t2s = spool.tile([NB, GROUP], bf16)
            for h in range(2):
                nc.vector.scalar_tensor_tensor(
                    out=t2s[:, h * HALF : (h + 1) * HALF],
                    in0=yt[:, h * BANK : h * BANK + HALF],
                    scalar=1.0,
                    in1=a_sb[:, g * GROUP + h * HALF : g * GROUP + (h + 1) * HALF],
                    op0=mybir.AluOpType.mult,
                    op1=mybir.AluOpType.mult,
                    accum_out=acc_t2[:, 2 * g + h : 2 * g + h + 1],
                )

            # T1 partials: x^2 split between DVE (fused) and ACT (Square)
            xf = xt[:].rearrange("p c f -> p (c f)")
            sqs = spool.tile([KL, DVE_SQ], bf16)
            nc.vector.scalar_tensor_tensor(
                out=sqs[:],
                in0=xf[:, :DVE_SQ],
                scalar=1.0,
                in1=xf[:, :DVE_SQ],
                op0=mybir.AluOpType.mult,
                op1=mybir.AluOpType.mult,
                accum_out=acc_sq[:, NG + g : NG + g + 1],
            )
            sqa = spool.tile([KL, XCOLS - DVE_SQ], bf16)
            nc.scalar.activation(
                sqa[:],
                xf[:, DVE_SQ:],
                mybir.ActivationFunctionType.Square,
                accum_out=acc_sq[:, g : g + 1],
            )

            if g == 1:
                # probe: TT mult (bf16 2x?) + tensor_reduce speeds
                nc.vector.tensor_tensor(
                    pscr[:], xf[:], xf[:], mybir.AluOpType.mult
                )
                nc.vector.tensor_reduce(
                    probe[:, 0:1], pscr[:], mybir.AxisListType.X, mybir.AluOpType.add
                )
                # probe: bn_stats (6 windows of 500)
                nc.vector.bn_stats(probe[:, 8:14], xf[:, :HALF])
            if g == 3:
                # probe: gpsimd elementwise rate
                nc.gpsimd.tensor_tensor(
                    pscr[:, :XCOLS], xf[:], xf[:], mybir.AluOpType.mult
                )
                nc.gpsimd.tensor_reduce(
                    probe[0:1, 4:5],
                    pscr[:, :XCOLS],
                    mybir.AxisListType.XYZWC,
                    mybir.AluOpType.add,
                )

        nc.sync.dma_start(out_sq[:], acc_sq[:])
        nc.sync.dma_start(out_t2[:], acc_t2[:])

    nc.compile()
    _NC = nc
    return nc


def _make_in_maps(x, beta, A):
    import ml_dtypes

    bf16 = ml_dtypes.bfloat16
    x = np.asarray(x, dtype=np.float32)
    beta = np.asarray(beta, dtype=np.float32)
    A = np.asarray(A, dtype=np.float32)

    # (KL, C, cores, NG, GROUP) -> (cores, NG, KL, C, GROUP)
    xr = x.reshape(KL, C, N_CORES, NG, GROUP).transpose(2, 3, 0, 1, 4)
    xb = np.ascontiguousarray(xr.astype(bf16))
    bt = np.ascontiguousarray(beta.reshape(KL, NB).astype(bf16))
    # A^T shards: (cores, NB, HW_SHARD)
    at = np.ascontiguousarray(
        A.reshape(N_CORES, HW_SHARD, NB).transpose(0, 2, 1).astype(bf16)
    )

    in_maps = []
    for i in range(N_CORES):
        in_maps.append(
            {
                "xg": np.ascontiguousarray(xb[i]),
                "bsb": bt,
                "asb": at[i],
            }
        )
    return in_maps


def _run(in_maps, trace=False, **kwargs):
    from concourse import bass_utils

    nc = _build()
    return bass_utils.run_bass_kernel_spmd(
        nc, in_maps, list(range(N_CORES)), trace=trace, **kwargs
    )


def _combine(results, beta, A):
    t1 = 0.0
    t2 = 0.0
    for r in results:
        t1 += float(np.sum(np.asarray(r["out_sq"], dtype=np.float64)))
        t2 += float(np.sum(np.asarray(r["out_t2"], dtype=np.float64)))
    bf = np.asarray(beta, dtype=np.float64).reshape(KL, NB)
    af = np.asarray(A, dtype=np.float64).reshape(HW, NB)
    m = af.T @ af  # 3x3
    t3 = float(C) * SCALE * SCALE * float(np.einsum("kb,bc,kc->", bf, m, bf))
    total = t1 - 2.0 * SCALE * t2 + t3
    return np.float32(total / DENOM)


def kernel(x, beta, A):
    res = _run(_make_in_maps(x, beta, A))
    return _combine(res.results, beta, A)


# revision 12
# speedup vs baseline: 1.7859x; 1.7859x over previous
"""Trainium2 Bass kernel for the CNN-VAE loss:

    prob = einsum('klb,hwb->klhw', beta, A) * 5000
    mse  = mean(sum(|x - prob[:, :, None]|^2, axis=1))

Strategy (v5: algebraic expansion, bf16 stream)
-----------------------------------------------
Expand  sum |x - p|^2 = sum x^2 - 2*sum x*p + C*sum p^2  (p broadcast over
the C=3 channel dim).  With p = SCALE * einsum('klb,hwb', beta, A):

  T1 = sum x^2            -> device; split between ACT Square+accum_out and
       DVE fused scalar_tensor_tensor (x*x with accum_out) so both engines
       stay inside the per-group DMA window.
  T2 = -2*SCALE * sum_b sum_pix A[pix,b]*Y[b,pix],
       Y[b,pix] = sum_{kl,c} beta[kl,b]*x[kl,c,pix]
       -> Y on the PE: beta (128x3 bf16) stationary, x streaming, the c-sum
       folded into the PSUM accumulation (3 matmuls per 500-px chunk).  The
       A-weighted pixel reduction is one fused DVE scalar_tensor_tensor
       straight out of PSUM per chunk.
  T3 = C*SCALE^2 * sum_kl beta^T (A^T A) beta  -> host, f64, from the tiny
       beta/A inputs.

x streams as bf16 (halves HBM traffic; quantization contributes <1e-9
relative error vs the 2e-2 tolerance since T1+T2 are ~1e-7 of the total).
The hw axis is sharded across 8 cores (5000 px each).  Startup tricks:
beta rides inside group 0's x DMA (a separate descriptor queue lands ~5us
late), the ACT Square spline table and the PE HAM clock gate are warmed
with dummy work while the first DMA is in flight, and all group buffers
are resident simultaneously so every x DMA issues back-to-back.
"""

import numpy as np

K, L, NB, H, W = 16, 8, 3, 200, 200
KL = K * L          # 128 partitions
C = 3               # broadcast channel dim of x
HW = H * W          # 40000
N_CORES = 8
HW_SHARD = HW // N_CORES   # 5000
GROUP = 1000               # pixels per steady-state iteration
NG = HW_SHARD // GROUP     # 5
HALF = GROUP // 2          # 500: matmul free-dim chunk (fits one PSUM bank)
BANK = 512                 # PSUM bank width in f32
XCOLS = C * GROUP          # 3000 x columns per group
GW = XCOLS + 8             # group row width: x + embedded beta (3) + pad
DVE_SQ = 776               # x^2 columns handled by DVE (rest on ACT)
SCALE = 5000.0
DENOM = float(K * C * H * W)  # mean denominator (sum over L folded in)

_NC = None


def _build():
    global _NC
    if _NC is not None:
        return _NC
    from contextlib import ExitStack

    import concourse.bacc as bacc
    import concourse.mybir as mybir
    import concourse.tile as tile

    f32 = mybir.dt.float32
    bf16 = mybir.dt.bfloat16
    nc = bacc.Bacc("TRN2", target_bir_lowering=False, debug=False)

    xg = nc.dram_tensor("xg", [NG, KL, GW], bf16, kind="ExternalInput").ap()
    asb = nc.dram_tensor("asb", [NB, HW_SHARD], bf16, kind="ExternalInput").ap()
    out = nc.dram_tensor("out", [KL, 20], f32, kind="ExternalOutput").ap()

    with tile.TileContext(nc) as tc, ExitStack() as ctx:
        const = ctx.enter_context(tc.tile_pool(name="const", bufs=1))
        xpool = ctx.enter_context(tc.tile_pool(name="x", bufs=NG))
        spool = ctx.enter_context(tc.tile_pool(name="scr", bufs=2))
        ppool = ctx.enter_context(tc.tile_pool(name="psum", bufs=3, space="PSUM"))
        wpool = ctx.enter_context(tc.tile_pool(name="wps", bufs=1, space="PSUM"))

        # A^T on the GPSIMD SWDGE queue (not needed until ~13us; lands ~12)
        a_sb = const.tile([NB, HW_SHARD], bf16)
        nc.gpsimd.dma_start(a_sb[:], asb[:])

        acc = const.tile([KL, 20], f32)
        nc.vector.memset(acc[:], 0.0)

        # warm the ACT Square spline table while DMAs are in flight
        warm = const.tile([KL, 8], f32)
        nc.vector.memset(warm[:], 0.0)
        nc.scalar.activation(warm[:], warm[:], mybir.ActivationFunctionType.Square)

        # warm the PE HAM clock gate (idle default is half clock; ~3.4us of
        # activity lifts it before the real matmuls arrive)
        wmm = const.tile([KL, BANK], bf16)
        nc.vector.memset(wmm[:], 0.0)
        ydum = wpool.tile([NB, BANK], f32)
        for _ in range(12):
            nc.tensor.matmul(
                ydum[:, :BANK], wmm[:, :NB], wmm[:], start=True, stop=True
            )

        xts = []
        for g in range(NG):
            xt = xpool.tile([KL, GW], bf16)
            nc.sync.dma_start(xt[:], xg[g])
            xts.append(xt)
        b_sb = xts[0][:, XCOLS : XCOLS + NB]  # beta rides in group 0

        for g in range(NG):
            xt = xts[g]

            # PE: Y[b, pix] += sum_kl beta[kl,b]*x[kl,c,pix], c folded into
            # the PSUM accumulation group
            yt = ppool.tile([NB, 2 * BANK], f32)
            for h in range(2):
                for c in range(C):
                    nc.tensor.matmul(
                        yt[:, h * BANK : h * BANK + HALF],
                        b_sb,
                        xt[:, c * GROUP + h * HALF : c * GROUP + (h + 1) * HALF],
                        start=(c == 0),
                        stop=(c == C - 1),
                    )

            def emit_t2(g=g, xt=xt, yt=yt):
                t2s = spool.tile([NB, GROUP], bf16)
                for h in range(2):
                    nc.vector.scalar_tensor_tensor(
                        out=t2s[:, h * HALF : (h + 1) * HALF],
                        in0=yt[:, h * BANK : h * BANK + HALF],
                        scalar=1.0,
                        in1=a_sb[
                            :, g * GROUP + h * HALF : g * GROUP + (h + 1) * HALF
                        ],
                        op0=mybir.AluOpType.mult,
                        op1=mybir.AluOpType.mult,
                        accum_out=acc[0:NB, 10 + 2 * g + h : 11 + 2 * g + h],
                    )

            def emit_sq(g=g, xt=xt):
                sqs = spool.tile([KL, DVE_SQ], bf16)
                nc.vector.scalar_tensor_tensor(
                    out=sqs[:],
                    in0=xt[:, :DVE_SQ],
                    scalar=1.0,
                    in1=xt[:, :DVE_SQ],
                    op0=mybir.AluOpType.mult,
                    op1=mybir.AluOpType.mult,
                    accum_out=acc[:, 5 + g : 6 + g],
                )
                sqa = spool.tile([KL, XCOLS - DVE_SQ], bf16)
                nc.scalar.activation(
                    sqa[:],
                    xt[:, DVE_SQ:XCOLS],
                    mybir.ActivationFunctionType.Square,
                    accum_out=acc[:, g : g + 1],
                )

            if g == NG - 1:
                emit_sq()  # tail: squares don't depend on the matmuls
                emit_t2()
            else:
                emit_t2()
                emit_sq()

        nc.sync.dma_start(out[:], acc[:])

    nc.compile()
    _NC = nc
    return nc


def _make_in_maps(x, beta, A):
    import ml_dtypes

    bf16 = ml_dtypes.bfloat16
    x = np.asarray(x, dtype=np.float32)
    beta = np.asarray(beta, dtype=np.float32)
    A = np.asarray(A, dtype=np.float32)

    # (KL, C, cores, NG, GROUP) -> (cores, NG, KL, C*GROUP), plus the
    # embedded beta columns in group 0
    xr = x.reshape(KL, C, N_CORES, NG, GROUP).transpose(2, 3, 0, 1, 4)
    xgs = np.zeros((N_CORES, NG, KL, GW), dtype=bf16)
    xgs[:, :, :, :XCOLS] = xr.reshape(N_CORES, NG, KL, XCOLS).astype(bf16)
    xgs[:, 0, :, XCOLS : XCOLS + NB] = beta.reshape(KL, NB).astype(bf16)[None]
    # A^T shards: (cores, NB, HW_SHARD)
    at = np.ascontiguousarray(
        A.reshape(N_CORES, HW_SHARD, NB).transpose(0, 2, 1).astype(bf16)
    )

    in_maps = []
    for i in range(N_CORES):
        in_maps.append(
            {
                "xg": np.ascontiguousarray(xgs[i]),
                "asb": at[i],
            }
        )
    return in_maps


def _run(in_maps, trace=False, **kwargs):
    from concourse import bass_utils

    nc = _build()
    return bass_utils.run_bass_kernel_spmd(
        nc, in_maps, list(range(N_CORES)), trace=trace, **kwargs
    )


def _combine(results, beta, A):
    t1 = 0.0
    t2 = 0.0
    for r in results:
        o = np.asarray(r["out"], dtype=np.float64)
        t1 += float(np.sum(o[:, :10]))
        t2 += float(np.sum(o[:NB, 10:20]))
    bf = np.asarray(beta, dtype=np.float64).reshape(KL, NB)
    af = np.asarray(A, dtype=np.float64).reshape(HW, NB)
    m = af.T @ af  # 3x3
    t3 = float(C) * SCALE * SCALE * float(np.einsum("kb,bc,kc->", bf, m, bf))
    total = t1 - 2.0 * SCALE * t2 + t3
    return np.float32(total / DENOM)


def kernel(x, beta, A):
    res = _run(_make_in_maps(x, beta, A))
    return _combine(res.results, beta, A)
